# revision 32
# baseline (speedup 1.0000x reference)
"""Causal self-attention (RoPE) Trainium2 kernel, 8-core SPMD.

Sharding: core = (batch b, head-group g) -> 4 batches x 2 groups of 8 heads.
Each core: QKV projection for its 8 heads, RoPE, causal attention
(lazy softmax: unnormalized att @ [V|1] gives numerator + row-sums in one
matmul), then a partial output projection o^T = Wp_g^T @ y^T.
Host sums the two partial o^T per batch and adds b_proj.

Matmuls: float32r (fp32-data fast PE mode) except scores, which run in
bf16 on pair-layout copies (kt2/qt2) so each head's 64 dims sit on 64
contiguous partitions -> one K=64 matmul per score block, two heads
row-packed via tile_position.

Layouts (per core):
  xt  = x[b]^T [1024, 2048];  wq/wk [1024, 512] pi-permuted columns
  (chunks = [E h0-3, E h4-7, O h0-3, O h4-7], E/O = even/odd head dims) so
  RoPE pair-rotation is full-width [128, n] DVE ops across chunk pairs.
  wv [1024, 512] natural;  wp [512, 1024] rows = this core's head dims.
  kt2/qt2 (rotated, bf16) chunk m = heads (2m, 2m+1) x [E32|O32] each.
  v_sb [S, 520] f32r (65 cols/head: 64 v + ones col for the row-sum trick)
  scores transposed s^T[k, q]: out^T[d, q] = [V|1]^T @ exp(s^T), row 64 =
  softmax denominator.  Diagonal score blocks are N-trimmed to the causal
  range; one [128,128] lower-triangle mask handles the block diagonal.
"""

import os
import sys

sys.path.insert(0, "/opt/trn_rl_repo")

import numpy as np

B, S, C = 4, 2048, 1024
H, D = 16, 64
NC = 8
ROPE_BASE = 10000.0
SQ = 512          # q-chunk / streaming chunk
NJ = S // SQ      # 4

_BUILD_CACHE = {}


def _install_ntff_shim():
    """Optional: register antenv.axon_hooks so trace=True works under axon."""
    import contextlib
    import ctypes
    import types

    if "antenv.axon_hooks" in sys.modules:
        return
    so_path = "/opt/axon/libaxon_pjrt.so"
    if not os.path.exists(so_path):
        return
    lib = ctypes.CDLL(so_path)
    if not hasattr(lib, "axon_start_nrt_profile"):
        return
    lib.axon_start_nrt_profile.argtypes = [ctypes.POINTER(ctypes.c_int64), ctypes.c_size_t]
    lib.axon_start_nrt_profile.restype = ctypes.c_int64
    lib.axon_stop_nrt_profile.argtypes = [ctypes.c_char_p]
    lib.axon_stop_nrt_profile.restype = ctypes.c_int64

    @contextlib.contextmanager
    def _hook(output_dir, device_ids):
        import jax

        jax.devices()
        if device_ids:
            ids = (ctypes.c_int64 * len(device_ids))(*device_ids)
            rc = lib.axon_start_nrt_profile(ids, len(device_ids))
        else:
            rc = lib.axon_start_nrt_profile(None, 0)
        if rc != 0:
            raise RuntimeError(f"axon_start_nrt_profile rc={rc}")
        try:
            yield
        finally:
            lib.axon_stop_nrt_profile(output_dir.encode())

    mod = types.ModuleType("antenv.axon_hooks")
    mod.get_axon_ntff_profile_hook = lambda: _hook
    mod.set_axon_ntff_profile_hook = lambda h: None
    sys.modules["antenv.axon_hooks"] = mod


def _build():
    if "nc" in _BUILD_CACHE:
        return _BUILD_CACHE["nc"]
    import concourse.bacc as bacc
    import concourse.mybir as mybir
    from concourse.tile import TileContext

    f32 = mybir.dt.float32
    f32r = mybir.dt.float32r
    bf16 = mybir.dt.bfloat16
    ALU = mybir.AluOpType
    ACTF = mybir.ActivationFunctionType

    nc = bacc.Bacc("TRN2", target_bir_lowering=False, debug=False, num_devices=NC)

    xt_d = nc.dram_tensor("xt", [C, S], f32r, kind="ExternalInput")
    wq_d = nc.dram_tensor("wq", [C, 512], f32r, kind="ExternalInput")
    wk_d = nc.dram_tensor("wk", [C, 512], f32r, kind="ExternalInput")
    wv_d = nc.dram_tensor("wv", [C, 512], f32r, kind="ExternalInput")
    wp_d = nc.dram_tensor("wp", [512, C], f32r, kind="ExternalInput")
    bq_d = nc.dram_tensor("bq", [128, 4], f32, kind="ExternalInput")
    bk_d = nc.dram_tensor("bk", [128, 4], f32, kind="ExternalInput")
    bvr_d = nc.dram_tensor("bvr", [128, 512], f32, kind="ExternalInput")
    rope_d = nc.dram_tensor("rope", [4, 128, S], f32, kind="ExternalInput")
    tri_d = nc.dram_tensor("tri", [128, 128], f32r, kind="ExternalInput")
    o_d = nc.dram_tensor("o", [C, S], f32, kind="ExternalOutput")
    dbg = bool(os.environ.get("KSA_DEBUG"))
    if dbg:
        kt2_dbg = nc.dram_tensor("kt2_dbg", [128, 4, S], bf16, kind="ExternalOutput")
        v_dbg = nc.dram_tensor("v_dbg", [128, S // 128, 520], f32r, kind="ExternalOutput")
        qt2_dbg = nc.dram_tensor("qt2_dbg", [128, 4, SQ], bf16, kind="ExternalOutput")
        att_dbg = nc.dram_tensor("att_dbg", [128, 2, SQ], f32r, kind="ExternalOutput")
        av_dbg = nc.dram_tensor("av_dbg", [65, SQ], f32, kind="ExternalOutput")
        rec_dbg = nc.dram_tensor("rec_dbg", [64, SQ], f32, kind="ExternalOutput")
        yt_dbg = nc.dram_tensor("yt_dbg", [128, 4, SQ], f32r, kind="ExternalOutput")

    with TileContext(nc) as tc, tc.tile_pool(name="const", bufs=1) as constp:
        wq_sb = constp.tile([128, 8, 512], f32r, name="wq_sb", tag="wq")
        wk_sb = constp.tile([128, 8, 512], f32r, name="wk_sb", tag="wk")
        wv_sb = constp.tile([128, 8, 512], f32r, name="wv_sb", tag="wv")
        wp_sb = constp.tile([128, 4, C], f32r, name="wp_sb", tag="wp")
        bq_sb = constp.tile([128, 4], f32, name="bq_sb", tag="bq")
        bk_sb = constp.tile([128, 4], f32, name="bk_sb", tag="bk")
        bvr_sb = constp.tile([128, 512], f32, name="bvr_sb", tag="bvr")
        tri_sb = constp.tile([128, 128], f32r, name="tri_sb", tag="tri")
        ones1 = constp.tile([1, 64], f32r, name="ones1", tag="ones1")
        kt2 = constp.tile([128, 4, S], bf16, name="kt2", tag="kt2")
        v_sb = constp.tile([128, S // 128, 520], f32r, name="v_sb", tag="v")

        nc.sync.dma_start(out=wq_sb[:], in_=wq_d.rearrange("(c p) d -> p c d", p=128))
        nc.sync.dma_start(out=wk_sb[:], in_=wk_d.rearrange("(c p) d -> p c d", p=128))
        nc.sync.dma_start(out=wv_sb[:], in_=wv_d.rearrange("(c p) d -> p c d", p=128))
        nc.sync.dma_start(out=wp_sb[:], in_=wp_d.rearrange("(c p) o -> p c o", p=128))
        nc.sync.dma_start(out=bq_sb[:], in_=bq_d[:])
        nc.sync.dma_start(out=bk_sb[:], in_=bk_d[:])
        nc.sync.dma_start(out=bvr_sb[:], in_=bvr_d[:])
        nc.sync.dma_start(out=tri_sb[:], in_=tri_d[:])
        nc.vector.memset(ones1[:].bitcast(f32), 1.0)
        for hh in range(8):
            nc.vector.memset(v_sb[:, :, 65 * hh + 64 : 65 * hh + 65].bitcast(f32), 1.0)

        with (
            tc.tile_pool(name="xtp", bufs=2) as xtp,
            tc.tile_pool(name="ropep", bufs=1) as ropep,
            tc.tile_pool(name="scrp", bufs=6) as scrp,
            tc.tile_pool(name="eop", bufs=3) as eop,
            tc.tile_pool(name="qt2p", bufs=2) as qt2p,
            tc.tile_pool(name="attp", bufs=2) as attp,
            tc.tile_pool(name="yp", bufs=1) as yp,
            tc.tile_pool(name="otp", bufs=2) as otp,
            tc.tile_pool(name="psp", bufs=3, space="PSUM") as psp,
            tc.tile_pool(name="avp", bufs=2, space="PSUM") as avp,
        ):
            for j in range(NJ):
                s0 = j * SQ
                # ---- phase A: QKV + RoPE --------------------------------
                xt_sb = xtp.tile([128, 8, SQ], f32r, tag="xt")
                nc.sync.dma_start(
                    out=xt_sb[:],
                    in_=xt_d.rearrange("(c p) s -> p c s", p=128)[:, :, s0 : s0 + SQ],
                )
                rp = ropep.tile([128, 4, SQ], f32, tag="rope")
                nc.sync.dma_start(
                    out=rp[:],
                    in_=rope_d.rearrange("t p s -> p t s")[:, :, s0 : s0 + SQ],
                )
                qt_eo = eop.tile([128, 4, SQ], bf16, tag="eo", name="qt_eo")
                kt_eo = eop.tile([128, 4, SQ], bf16, tag="eo", name="kt_eo")
                # cc pairs: psum tile holds (E chunk cc, O chunk cc)
                for (src_w, bias, eo_t) in (
                    (wq_sb, bq_sb, qt_eo),
                    (wk_sb, bk_sb, kt_eo),
                ):
                    for cc in range(2):
                        pch = [cc, 2 + cc]  # pi chunk indices (E_cc, O_cc)
                        ps = psp.tile([128, 2, SQ], f32, tag="ps", name="qk_ps")
                        for sl in range(2):
                            dcol = 128 * pch[sl]
                            for c in range(8):
                                nc.tensor.matmul(
                                    ps[:, sl, :],
                                    src_w[:, c, dcol : dcol + 128],
                                    xt_sb[:, c, :],
                                    start=(c == 0),
                                    stop=(c == 7),
                                )
                        # rotE = (E+b)ce - (O+b)se ; rotO = (O+b)co + (E+b)so
                        t1 = scrp.tile([128, SQ], f32, tag="scr")
                        t2 = scrp.tile([128, SQ], f32, tag="scr")
                        t3 = scrp.tile([128, SQ], f32, tag="scr")
                        t4 = scrp.tile([128, SQ], f32, tag="scr")
                        nc.vector.scalar_tensor_tensor(
                            t1[:], ps[:, 0, :], bias[:, pch[0] : pch[0] + 1],
                            rp[:, 0, :], ALU.add, ALU.mult,
                        )
                        nc.vector.scalar_tensor_tensor(
                            t2[:], ps[:, 1, :], bias[:, pch[1] : pch[1] + 1],
                            rp[:, 1, :], ALU.add, ALU.mult,
                        )
                        nc.vector.scalar_tensor_tensor(
                            t3[:], ps[:, 1, :], bias[:, pch[1] : pch[1] + 1],
                            rp[:, 2, :], ALU.add, ALU.mult,
                        )
                        nc.vector.scalar_tensor_tensor(
                            t4[:], ps[:, 0, :], bias[:, pch[0] : pch[0] + 1],
                            rp[:, 3, :], ALU.add, ALU.mult,
                        )
                        nc.gpsimd.tensor_sub(eo_t[:, pch[0], :], t1[:], t2[:])
                        nc.gpsimd.tensor_add(eo_t[:, pch[1], :], t3[:], t4[:])
                # v projection (natural layout, [s,d], 128-row subchunks)
                for uu in range(2):
                    v_ps = psp.tile([128, 2, SQ], f32, tag="ps", name="v_ps")
                    for u in range(2):
                        for c in range(8):
                            nc.tensor.matmul(
                                v_ps[:, u, :],
                                xt_sb[:, c, 256 * uu + 128 * u : 256 * uu + 128 * u + 128],
                                wv_sb[:, c, :],
                                start=(c == 0),
                                stop=(c == 7),
                            )
                    for u in range(2):
                        ch = 4 * j + 2 * uu + u
                        dst = v_sb[:, ch].rearrange("p (h d) -> p h d", d=65)[:, :, 0:64]
                        nc.vector.tensor_tensor(dst, v_ps[:, u, :], bvr_sb[:], ALU.add)
                # pair-layout copies (E/O chunks -> per-head 64-row bands)
                qt2 = qt2p.tile([128, 4, SQ], bf16, tag="qt2")
                for hh in range(8):
                    se_p, se_c = 32 * (hh % 4), hh // 4
                    de_p, de_c = 64 * (hh % 2), hh // 2
                    nc.sync.dma_start(
                        out=qt2[de_p : de_p + 32, de_c, :],
                        in_=qt_eo[se_p : se_p + 32, se_c, :],
                    )
                    nc.sync.dma_start(
                        out=qt2[de_p + 32 : de_p + 64, de_c, :],
                        in_=qt_eo[se_p : se_p + 32, 2 + se_c, :],
                    )
                    nc.sync.dma_start(
                        out=kt2[de_p : de_p + 32, de_c, s0 : s0 + SQ],
                        in_=kt_eo[se_p : se_p + 32, se_c, :],
                    )
                    nc.sync.dma_start(
                        out=kt2[de_p + 32 : de_p + 64, de_c, s0 : s0 + SQ],
                        in_=kt_eo[se_p : se_p + 32, 2 + se_c, :],
                    )

                if dbg and j == 0:
                    nc.sync.dma_start(out=qt2_dbg[:], in_=qt2[:])
                # ---- phase B: attention ---------------------------------
                nkb = 4 * j + 4
                yt = yp.tile([128, 4, SQ], f32r, tag="yt")
                for p in range(4):
                    avs = [
                        avp.tile([65, SQ], f32, tag="av", name=f"av{j}_{p}_{t}")
                        for t in range(2)
                    ]
                    for c in range(nkb):
                        off = 128 * (c - 4 * j) if c >= 4 * j else 0
                        sc = psp.tile([128, 2, SQ], f32, tag="ps", name="sc")
                        for t in range(2):
                            hi = 2 * p + t
                            rb, ch = 64 * (hi % 2), hi // 2
                            nc.tensor.matmul(
                                sc[:, t, off:SQ],
                                kt2[rb : rb + 64, ch, 128 * c : 128 * c + 128],
                                qt2[rb : rb + 64, ch, off:SQ],
                                start=True,
                                stop=True,
                                tile_position=(rb, 0),
                            )
                        att = attp.tile([128, 2, SQ], f32r, tag="att")
                        nc.scalar.activation(
                            att[:, :, off:SQ], sc[:, :, off:SQ], ACTF.Exp, scale=0.125
                        )
                        if c >= 4 * j:
                            for t in range(2):
                                nc.gpsimd.tensor_tensor(
                                    att[:, t, off : off + 128],
                                    att[:, t, off : off + 128],
                                    tri_sb[:],
                                    ALU.mult,
                                )
                        if dbg and j == 0 and p == 0 and c == 0:
                            nc.sync.dma_start(out=att_dbg[:], in_=att[:])
                        for t in range(2):
                            hi = 2 * p + t
                            nc.tensor.matmul(
                                avs[t][0:65, off:SQ],
                                v_sb[:, c, 65 * hi : 65 * hi + 65],
                                att[:, t, off:SQ],
                                start=(c == 0),
                                stop=(c == nkb - 1),
                                skip_group_check=True,
                            )
                    if dbg and j == 0 and p == 0:
                        av_cp = otp.tile([65, SQ], f32, tag="ot", name="av_cp")
                        nc.vector.tensor_copy(av_cp[:], avs[0][:])
                        nc.sync.dma_start(out=av_dbg[:], in_=av_cp[:])
                    bc = psp.tile([64, 2, SQ], f32, tag="ps", name="bc")
                    yus = []
                    for t in range(2):
                        yu = scrp.tile([65, SQ], f32, tag="scr", name="yu")
                        den0 = scrp.tile([1, SQ], f32, tag="scr", name="den0")
                        rec_f = scrp.tile([1, SQ], f32, tag="scr", name="rec_f")
                        rec_r = scrp.tile([1, SQ], f32r, tag="scr", name="rec_r")
                        nc.scalar.copy(yu[:], avs[t][:])
                        nc.vector.tensor_copy(den0[:], avs[t][64:65, :])
                        nc.vector.reciprocal_approx_fast(rec_f[:], den0[:])
                        nc.vector.tensor_copy(rec_r[:], rec_f[:])
                        nc.tensor.matmul(
                            bc[:, t, :], ones1[:], rec_r[:], start=True, stop=True,
                        )
                        yus.append(yu)
                    for t in range(2):
                        hi = 2 * p + t
                        nc.vector.tensor_mul(
                            yt[64 * (hi % 2) : 64 * (hi % 2) + 64, hi // 2, :],
                            yus[t][0:64, :],
                            bc[:, t, :],
                        )
                        if dbg and j == 0 and p == 0 and t == 0:
                            nc.sync.dma_start(out=rec_dbg[:], in_=rec64[:])

                if dbg and j == 0:
                    nc.sync.dma_start(out=yt_dbg[:], in_=yt[:])
                # ---- phase C: partial projection ------------------------
                for co in range(8):
                    pj = psp.tile([128, SQ], f32, tag="ps", name="pj")
                    for cin in range(4):
                        nc.tensor.matmul(
                            pj[:],
                            wp_sb[:, cin, 128 * co : 128 * co + 128],
                            yt[:, cin, :],
                            start=(cin == 0),
                            stop=(cin == 3),
                        )
                    ot = otp.tile([128, SQ], f32, tag="ot")
                    nc.vector.tensor_copy(ot[:], pj[:])
                    nc.sync.dma_start(
                        out=o_d[128 * co : 128 * co + 128, s0 : s0 + SQ],
                        in_=ot[:],
                    )

        if dbg:
            nc.sync.dma_start(out=kt2_dbg[:], in_=kt2[:])
            nc.sync.dma_start(out=v_dbg[:], in_=v_sb[:])

    nc.compile()
    _BUILD_CACHE["nc"] = nc
    return nc


def _host_prep(x, W_qkv, b_qkv, W_proj):
    """Build the 8 per-core input maps."""
    inv_freq = (1.0 / (ROPE_BASE ** (np.arange(0, D, 2, dtype=np.float32) / D))).astype(
        np.float32
    )
    pos = np.arange(S, dtype=np.float32)
    freqs = np.outer(pos, inv_freq).astype(np.float32)  # [S, 32]
    emb = np.concatenate([freqs, freqs], axis=-1)  # [S, 64]
    cos_t = np.cos(emb).astype(np.float32)
    sin_t = np.sin(emb).astype(np.float32)
    m = np.arange(32)
    ce = np.ascontiguousarray(cos_t[:, 2 * m].T)  # [32, S]
    se = np.ascontiguousarray(sin_t[:, 2 * m].T)
    co = np.ascontiguousarray(cos_t[:, 2 * m + 1].T)
    so = np.ascontiguousarray(sin_t[:, 2 * m + 1].T)
    rope = np.stack([np.tile(t, (4, 1)) for t in (ce, se, co, so)], axis=0)

    qq = np.arange(128)[None, :]
    rr = np.arange(128)[:, None]
    tri = (rr <= qq).astype(np.float32)

    in_maps = []
    for core in range(NC):
        b, g = core // 2, core % 2
        heads = list(range(8 * g, 8 * g + 8))
        cols_e = [64 * heads[4 * c4 + i] + 2 * mm
                  for c4 in range(2) for i in range(4) for mm in range(32)]
        cols_o = [64 * heads[4 * c4 + i] + 2 * mm + 1
                  for c4 in range(2) for i in range(4) for mm in range(32)]
        pi = np.array(cols_e + cols_o)
        vcols = np.array([64 * h + d for h in heads for d in range(64)])

        wq = np.ascontiguousarray(W_qkv[:, pi])
        wk = np.ascontiguousarray(W_qkv[:, C + pi])
        wv = np.ascontiguousarray(W_qkv[:, 2 * C + vcols])
        wp = np.ascontiguousarray(W_proj[vcols, :])
        bq = np.ascontiguousarray(b_qkv[pi].reshape(4, 128).T)
        bk = np.ascontiguousarray(b_qkv[C + pi].reshape(4, 128).T)
        bvr = np.tile(b_qkv[2 * C + vcols][None, :], (128, 1))
        xt = np.ascontiguousarray(x[b].T)
        in_maps.append(
            dict(xt=xt, wq=wq, wk=wk, wv=wv, wp=wp, bq=bq, bk=bk,
                 bvr=np.ascontiguousarray(bvr), rope=rope, tri=tri)
        )
    return in_maps


def kernel(x, W_qkv, b_qkv, W_proj, b_proj):
    from concourse.bass_utils import run_bass_kernel_spmd
    import concourse.bass_utils as bass_utils

    x = np.asarray(x, dtype=np.float32)
    W_qkv = np.asarray(W_qkv, dtype=np.float32)
    b_qkv = np.asarray(b_qkv, dtype=np.float32)
    W_proj = np.asarray(W_proj, dtype=np.float32)
    b_proj = np.asarray(b_proj, dtype=np.float32)

    trace = bool(os.environ.get("BASS_KERNEL_TRACE"))
    if trace:
        _install_ntff_shim()
        bass_utils.upload_artifacts = lambda tmpdir: "local://" + tmpdir

    nc = _build()
    in_maps = _host_prep(x, W_qkv, b_qkv, W_proj)
    kw = {}
    if trace:
        tdir = os.environ.get("BASS_KERNEL_TRACE_DIR", "/tmp/ksa_trace")
        os.makedirs(tdir, exist_ok=True)
        kw = dict(trace=True, tmpdir=tdir)
    try:
        res = run_bass_kernel_spmd(nc, in_maps, core_ids=list(range(NC)), **kw)
    except Exception:
        # transient NRT_EXEC_UNIT_UNRECOVERABLE has been observed on the
        # first execution after a fresh compile; one retry clears it
        res = run_bass_kernel_spmd(nc, in_maps, core_ids=list(range(NC)), **kw)
    _BUILD_CACHE["last_result"] = res

    out = np.empty((B, S, C), np.float32)
    for b in range(B):
        oT = res.results[2 * b]["o"] + res.results[2 * b + 1]["o"]
        out[b] = oT.T + b_proj[None, :]
    return out


# revision 34
# speedup vs baseline: 1.2578x; 1.2578x over previous
"""Causal self-attention (RoPE) Trainium2 kernel, 8-core SPMD.

Sharding: core = (batch b, head-group g) -> 4 batches x 2 groups of 8 heads.
Each core: QKV projection for its 8 heads, RoPE, causal attention
(lazy softmax: unnormalized att @ [V|1] gives numerator + row-sums in one
matmul), then a partial output projection o^T = Wp_g^T @ y^T.
Host sums the two partial o^T per batch and adds b_proj.

Matmuls: float32r (fp32-data fast PE mode) except scores, which run in
bf16 on pair-layout copies (kt2/qt2) so each head's 64 dims sit on 64
contiguous partitions -> one K=64 matmul per score block, two heads
row-packed via tile_position.

Layouts (per core):
  xt  = x[b]^T [1024, 2048];  wq/wk [1024, 512] pi-permuted columns
  (chunks = [E h0-3, E h4-7, O h0-3, O h4-7], E/O = even/odd head dims) so
  RoPE pair-rotation is full-width [128, n] DVE ops across chunk pairs.
  wv [1024, 512] natural;  wp [512, 1024] rows = this core's head dims.
  kt2/qt2 (rotated, bf16) chunk m = heads (2m, 2m+1) x [E32|O32] each.
  v_sb [S, 520] f32r (65 cols/head: 64 v + ones col for the row-sum trick)
  scores transposed s^T[k, q]: out^T[d, q] = [V|1]^T @ exp(s^T), row 64 =
  softmax denominator.  Diagonal score blocks are N-trimmed to the causal
  range; one [128,128] lower-triangle mask handles the block diagonal.
"""

import os
import sys

sys.path.insert(0, "/opt/trn_rl_repo")

import numpy as np
import ml_dtypes

B, S, C = 4, 2048, 1024
H, D = 16, 64
NC = 8
ROPE_BASE = 10000.0
SQ = 512          # q-chunk / streaming chunk
NJ = S // SQ      # 4

_BUILD_CACHE = {}


def _install_ntff_shim():
    """Optional: register antenv.axon_hooks so trace=True works under axon."""
    import contextlib
    import ctypes
    import types

    if "antenv.axon_hooks" in sys.modules:
        return
    so_path = "/opt/axon/libaxon_pjrt.so"
    if not os.path.exists(so_path):
        return
    lib = ctypes.CDLL(so_path)
    if not hasattr(lib, "axon_start_nrt_profile"):
        return
    lib.axon_start_nrt_profile.argtypes = [ctypes.POINTER(ctypes.c_int64), ctypes.c_size_t]
    lib.axon_start_nrt_profile.restype = ctypes.c_int64
    lib.axon_stop_nrt_profile.argtypes = [ctypes.c_char_p]
    lib.axon_stop_nrt_profile.restype = ctypes.c_int64

    @contextlib.contextmanager
    def _hook(output_dir, device_ids):
        import jax

        jax.devices()
        if device_ids:
            ids = (ctypes.c_int64 * len(device_ids))(*device_ids)
            rc = lib.axon_start_nrt_profile(ids, len(device_ids))
        else:
            rc = lib.axon_start_nrt_profile(None, 0)
        if rc != 0:
            raise RuntimeError(f"axon_start_nrt_profile rc={rc}")
        try:
            yield
        finally:
            lib.axon_stop_nrt_profile(output_dir.encode())

    mod = types.ModuleType("antenv.axon_hooks")
    mod.get_axon_ntff_profile_hook = lambda: _hook
    mod.set_axon_ntff_profile_hook = lambda h: None
    sys.modules["antenv.axon_hooks"] = mod


def _build():
    if "nc" in _BUILD_CACHE:
        return _BUILD_CACHE["nc"]
    import concourse.bacc as bacc
    import concourse.mybir as mybir
    from concourse.tile import TileContext

    f32 = mybir.dt.float32
    f32r = mybir.dt.float32r
    bf16 = mybir.dt.bfloat16
    ALU = mybir.AluOpType
    ACTF = mybir.ActivationFunctionType

    nc = bacc.Bacc("TRN2", target_bir_lowering=False, debug=False, num_devices=NC)

    xt_d = nc.dram_tensor("xt", [C, S], f32r, kind="ExternalInput")
    wq_d = nc.dram_tensor("wq", [C, 512], f32r, kind="ExternalInput")
    wk_d = nc.dram_tensor("wk", [C, 512], f32r, kind="ExternalInput")
    wv_d = nc.dram_tensor("wv", [C, 512], f32r, kind="ExternalInput")
    wp_d = nc.dram_tensor("wp", [512, C], f32r, kind="ExternalInput")
    bq_d = nc.dram_tensor("bq", [128, 4], f32, kind="ExternalInput")
    bk_d = nc.dram_tensor("bk", [128, 4], f32, kind="ExternalInput")
    bvr_d = nc.dram_tensor("bvr", [128, 512], f32, kind="ExternalInput")
    rope_d = nc.dram_tensor("rope", [4, 128, S], f32, kind="ExternalInput")
    tri_d = nc.dram_tensor("tri", [128, 128], bf16, kind="ExternalInput")
    o_d = nc.dram_tensor("o", [C, S], f32, kind="ExternalOutput")
    dbg = bool(os.environ.get("KSA_DEBUG"))
    if dbg:
        kt2_dbg = nc.dram_tensor("kt2_dbg", [128, 4, S], bf16, kind="ExternalOutput")
        v_dbg = nc.dram_tensor("v_dbg", [128, S // 128, 520], f32r, kind="ExternalOutput")
        qt2_dbg = nc.dram_tensor("qt2_dbg", [128, 4, SQ], bf16, kind="ExternalOutput")
        att_dbg = nc.dram_tensor("att_dbg", [128, 2, SQ], f32r, kind="ExternalOutput")
        av_dbg = nc.dram_tensor("av_dbg", [65, SQ], f32, kind="ExternalOutput")
        rec_dbg = nc.dram_tensor("rec_dbg", [64, SQ], f32, kind="ExternalOutput")
        yt_dbg = nc.dram_tensor("yt_dbg", [128, 4, SQ], f32r, kind="ExternalOutput")

    with TileContext(nc) as tc, tc.tile_pool(name="const", bufs=1) as constp:
        wq_sb = constp.tile([128, 8, 512], f32r, name="wq_sb", tag="wq")
        wk_sb = constp.tile([128, 8, 512], f32r, name="wk_sb", tag="wk")
        wv_sb = constp.tile([128, 8, 512], f32r, name="wv_sb", tag="wv")
        wp_sb = constp.tile([128, 4, C], f32r, name="wp_sb", tag="wp")
        bq_sb = constp.tile([128, 4], f32, name="bq_sb", tag="bq")
        bk_sb = constp.tile([128, 4], f32, name="bk_sb", tag="bk")
        bvr_sb = constp.tile([128, 512], f32, name="bvr_sb", tag="bvr")
        tri_sb = constp.tile([128, 128], bf16, name="tri_sb", tag="tri")
        ones1 = constp.tile([1, 64], f32r, name="ones1", tag="ones1")
        kt2 = constp.tile([128, 4, S], bf16, name="kt2", tag="kt2")
        v_sb = constp.tile([128, S // 128, 520], bf16, name="v_sb", tag="v")

        nc.sync.dma_start(out=wq_sb[:], in_=wq_d.rearrange("(c p) d -> p c d", p=128))
        nc.sync.dma_start(out=wk_sb[:], in_=wk_d.rearrange("(c p) d -> p c d", p=128))
        nc.sync.dma_start(out=wv_sb[:], in_=wv_d.rearrange("(c p) d -> p c d", p=128))
        nc.sync.dma_start(out=wp_sb[:], in_=wp_d.rearrange("(c p) o -> p c o", p=128))
        nc.sync.dma_start(out=bq_sb[:], in_=bq_d[:])
        nc.sync.dma_start(out=bk_sb[:], in_=bk_d[:])
        nc.sync.dma_start(out=bvr_sb[:], in_=bvr_d[:])
        nc.sync.dma_start(out=tri_sb[:], in_=tri_d[:])
        nc.vector.memset(ones1[:].bitcast(f32), 1.0)
        for hh in range(8):
            nc.vector.memset(v_sb[:, :, 65 * hh + 64 : 65 * hh + 65], 1.0)

        with (
            tc.tile_pool(name="xtp", bufs=2) as xtp,
            tc.tile_pool(name="ropep", bufs=2) as ropep,
            tc.tile_pool(name="scrp", bufs=6) as scrp,
            tc.tile_pool(name="eop", bufs=3) as eop,
            tc.tile_pool(name="qt2p", bufs=2) as qt2p,
            tc.tile_pool(name="attp", bufs=3) as attp,
            tc.tile_pool(name="yp", bufs=1) as yp,
            tc.tile_pool(name="otp", bufs=2) as otp,
            tc.tile_pool(name="psp", bufs=3, space="PSUM") as psp,
            tc.tile_pool(name="avp", bufs=2, space="PSUM") as avp,
        ):
            for j in range(NJ):
                s0 = j * SQ
                # ---- phase A: QKV + RoPE --------------------------------
                xt_sb = xtp.tile([128, 8, SQ], f32r, tag="xt")
                nc.sync.dma_start(
                    out=xt_sb[:],
                    in_=xt_d.rearrange("(c p) s -> p c s", p=128)[:, :, s0 : s0 + SQ],
                )
                rp = ropep.tile([128, 4, SQ], f32, tag="rope")
                nc.sync.dma_start(
                    out=rp[:],
                    in_=rope_d.rearrange("t p s -> p t s")[:, :, s0 : s0 + SQ],
                )
                qt_eo = eop.tile([128, 4, SQ], bf16, tag="eo", name="qt_eo")
                kt_eo = eop.tile([128, 4, SQ], bf16, tag="eo", name="kt_eo")
                # cc pairs: psum tile holds (E chunk cc, O chunk cc)
                for (src_w, bias, eo_t) in (
                    (wq_sb, bq_sb, qt_eo),
                    (wk_sb, bk_sb, kt_eo),
                ):
                    for cc in range(2):
                        pch = [cc, 2 + cc]  # pi chunk indices (E_cc, O_cc)
                        ps = psp.tile([128, 2, SQ], f32, tag="ps", name="qk_ps")
                        for sl in range(2):
                            dcol = 128 * pch[sl]
                            for c in range(8):
                                nc.tensor.matmul(
                                    ps[:, sl, :],
                                    src_w[:, c, dcol : dcol + 128],
                                    xt_sb[:, c, :],
                                    start=(c == 0),
                                    stop=(c == 7),
                                )
                        # rotE = (E+b)ce - (O+b)se ; rotO = (O+b)co + (E+b)so
                        t1 = scrp.tile([128, SQ], f32, tag="scr")
                        t2 = scrp.tile([128, SQ], f32, tag="scr")
                        t3 = scrp.tile([128, SQ], f32, tag="scr")
                        t4 = scrp.tile([128, SQ], f32, tag="scr")
                        nc.vector.scalar_tensor_tensor(
                            t1[:], ps[:, 0, :], bias[:, pch[0] : pch[0] + 1],
                            rp[:, 0, :], ALU.add, ALU.mult,
                        )
                        nc.vector.scalar_tensor_tensor(
                            t2[:], ps[:, 1, :], bias[:, pch[1] : pch[1] + 1],
                            rp[:, 1, :], ALU.add, ALU.mult,
                        )
                        nc.vector.scalar_tensor_tensor(
                            t3[:], ps[:, 1, :], bias[:, pch[1] : pch[1] + 1],
                            rp[:, 2, :], ALU.add, ALU.mult,
                        )
                        nc.vector.scalar_tensor_tensor(
                            t4[:], ps[:, 0, :], bias[:, pch[0] : pch[0] + 1],
                            rp[:, 3, :], ALU.add, ALU.mult,
                        )
                        nc.gpsimd.tensor_sub(eo_t[:, pch[0], :], t1[:], t2[:])
                        nc.gpsimd.tensor_add(eo_t[:, pch[1], :], t3[:], t4[:])
                # v projection (natural layout, [s,d], 128-row subchunks)
                for uu in range(2):
                    v_ps = psp.tile([128, 2, SQ], f32, tag="ps", name="v_ps")
                    for u in range(2):
                        for c in range(8):
                            nc.tensor.matmul(
                                v_ps[:, u, :],
                                xt_sb[:, c, 256 * uu + 128 * u : 256 * uu + 128 * u + 128],
                                wv_sb[:, c, :],
                                start=(c == 0),
                                stop=(c == 7),
                            )
                    for u in range(2):
                        ch = 4 * j + 2 * uu + u
                        dst = v_sb[:, ch].rearrange("p (h d) -> p h d", d=65)[:, :, 0:64]
                        nc.vector.tensor_tensor(dst, v_ps[:, u, :], bvr_sb[:], ALU.add)
                # pair-layout copies (E/O chunks -> per-head 64-row bands)
                qt2 = qt2p.tile([128, 4, SQ], bf16, tag="qt2")
                for hh in range(8):
                    se_p, se_c = 32 * (hh % 4), hh // 4
                    de_p, de_c = 64 * (hh % 2), hh // 2
                    nc.sync.dma_start(
                        out=qt2[de_p : de_p + 32, de_c, :],
                        in_=qt_eo[se_p : se_p + 32, se_c, :],
                    )
                    nc.sync.dma_start(
                        out=qt2[de_p + 32 : de_p + 64, de_c, :],
                        in_=qt_eo[se_p : se_p + 32, 2 + se_c, :],
                    )
                    nc.sync.dma_start(
                        out=kt2[de_p : de_p + 32, de_c, s0 : s0 + SQ],
                        in_=kt_eo[se_p : se_p + 32, se_c, :],
                    )
                    nc.sync.dma_start(
                        out=kt2[de_p + 32 : de_p + 64, de_c, s0 : s0 + SQ],
                        in_=kt_eo[se_p : se_p + 32, 2 + se_c, :],
                    )

                if dbg and j == 0:
                    nc.sync.dma_start(out=qt2_dbg[:], in_=qt2[:])
                # ---- phase B: attention ---------------------------------
                nkb = 4 * j + 4
                yt = yp.tile([128, 4, SQ], f32r, tag="yt")
                for p in range(4):
                    avs = [
                        avp.tile([65, SQ], f32, tag="av", name=f"av{j}_{p}_{t}")
                        for t in range(2)
                    ]
                    for c in range(nkb):
                        off = 128 * (c - 4 * j) if c >= 4 * j else 0
                        sc = psp.tile([128, 2, SQ], f32, tag="ps", name="sc")
                        for t in range(2):
                            hi = 2 * p + t
                            rb, ch = 64 * (hi % 2), hi // 2
                            nc.tensor.matmul(
                                sc[:, t, off:SQ],
                                kt2[rb : rb + 64, ch, 128 * c : 128 * c + 128],
                                qt2[rb : rb + 64, ch, off:SQ],
                                start=True,
                                stop=True,
                                tile_position=(rb, 0),
                            )
                        att = attp.tile([128, 2, SQ], bf16, tag="att")
                        nc.scalar.activation(
                            att[:, :, off:SQ], sc[:, :, off:SQ], ACTF.Exp, scale=0.125
                        )
                        if c >= 4 * j:
                            for t in range(2):
                                nc.gpsimd.tensor_tensor(
                                    att[:, t, off : off + 128],
                                    att[:, t, off : off + 128],
                                    tri_sb[:],
                                    ALU.mult,
                                )
                        if dbg and j == 0 and p == 0 and c == 0:
                            nc.sync.dma_start(out=att_dbg[:], in_=att[:])
                        for t in range(2):
                            hi = 2 * p + t
                            nc.tensor.matmul(
                                avs[t][0:65, off:SQ],
                                v_sb[:, c, 65 * hi : 65 * hi + 65],
                                att[:, t, off:SQ],
                                start=(c == 0),
                                stop=(c == nkb - 1),
                                skip_group_check=True,
                            )
                    if dbg and j == 0 and p == 0:
                        av_cp = otp.tile([65, SQ], f32, tag="ot", name="av_cp")
                        nc.vector.tensor_copy(av_cp[:], avs[0][:])
                        nc.sync.dma_start(out=av_dbg[:], in_=av_cp[:])
                    for t in range(2):
                        hi = 2 * p + t
                        yu = scrp.tile([65, SQ], f32, tag="scr", name="yu")
                        den0 = scrp.tile([1, SQ], f32, tag="scr", name="den0")
                        rec_f = scrp.tile([1, SQ], f32, tag="scr", name="rec_f")
                        rec_r = scrp.tile([1, SQ], f32r, tag="scr", name="rec_r")
                        nc.scalar.copy(yu[:], avs[t][:])
                        nc.vector.tensor_copy(den0[:], avs[t][64:65, :])
                        nc.vector.reciprocal_approx_fast(rec_f[:], den0[:])
                        nc.vector.tensor_copy(rec_r[:], rec_f[:])
                        bc = avp.tile([64, SQ], f32, tag="av", name="bc")
                        nc.tensor.matmul(bc[:], ones1[:], rec_r[:], start=True, stop=True)
                        nc.vector.tensor_mul(
                            yt[64 * (hi % 2) : 64 * (hi % 2) + 64, hi // 2, :],
                            yu[0:64, :],
                            bc[:],
                        )
                        if dbg and j == 0 and p == 0 and t == 0:
                            nc.sync.dma_start(out=rec_dbg[:], in_=rec64[:])

                if dbg and j == 0:
                    nc.sync.dma_start(out=yt_dbg[:], in_=yt[:])
                # ---- phase C: partial projection ------------------------
                for co in range(8):
                    pj = psp.tile([128, SQ], f32, tag="ps", name="pj")
                    for cin in range(4):
                        nc.tensor.matmul(
                            pj[:],
                            wp_sb[:, cin, 128 * co : 128 * co + 128],
                            yt[:, cin, :],
                            start=(cin == 0),
                            stop=(cin == 3),
                        )
                    ot = otp.tile([128, SQ], f32, tag="ot")
                    nc.vector.tensor_copy(ot[:], pj[:])
                    nc.sync.dma_start(
                        out=o_d[128 * co : 128 * co + 128, s0 : s0 + SQ],
                        in_=ot[:],
                    )

        if dbg:
            nc.sync.dma_start(out=kt2_dbg[:], in_=kt2[:])
            nc.sync.dma_start(out=v_dbg[:], in_=v_sb[:])

    nc.compile()
    _BUILD_CACHE["nc"] = nc
    return nc


def _host_prep(x, W_qkv, b_qkv, W_proj):
    """Build the 8 per-core input maps."""
    inv_freq = (1.0 / (ROPE_BASE ** (np.arange(0, D, 2, dtype=np.float32) / D))).astype(
        np.float32
    )
    pos = np.arange(S, dtype=np.float32)
    freqs = np.outer(pos, inv_freq).astype(np.float32)  # [S, 32]
    emb = np.concatenate([freqs, freqs], axis=-1)  # [S, 64]
    cos_t = np.cos(emb).astype(np.float32)
    sin_t = np.sin(emb).astype(np.float32)
    m = np.arange(32)
    ce = np.ascontiguousarray(cos_t[:, 2 * m].T)  # [32, S]
    se = np.ascontiguousarray(sin_t[:, 2 * m].T)
    co = np.ascontiguousarray(cos_t[:, 2 * m + 1].T)
    so = np.ascontiguousarray(sin_t[:, 2 * m + 1].T)
    rope = np.stack([np.tile(t, (4, 1)) for t in (ce, se, co, so)], axis=0)

    qq = np.arange(128)[None, :]
    rr = np.arange(128)[:, None]
    tri = (rr <= qq).astype(ml_dtypes.bfloat16)

    in_maps = []
    for core in range(NC):
        b, g = core // 2, core % 2
        heads = list(range(8 * g, 8 * g + 8))
        cols_e = [64 * heads[4 * c4 + i] + 2 * mm
                  for c4 in range(2) for i in range(4) for mm in range(32)]
        cols_o = [64 * heads[4 * c4 + i] + 2 * mm + 1
                  for c4 in range(2) for i in range(4) for mm in range(32)]
        pi = np.array(cols_e + cols_o)
        vcols = np.array([64 * h + d for h in heads for d in range(64)])

        wq = np.ascontiguousarray(W_qkv[:, pi])
        wk = np.ascontiguousarray(W_qkv[:, C + pi])
        wv = np.ascontiguousarray(W_qkv[:, 2 * C + vcols])
        wp = np.ascontiguousarray(W_proj[vcols, :])
        bq = np.ascontiguousarray(b_qkv[pi].reshape(4, 128).T)
        bk = np.ascontiguousarray(b_qkv[C + pi].reshape(4, 128).T)
        bvr = np.tile(b_qkv[2 * C + vcols][None, :], (128, 1))
        xt = np.ascontiguousarray(x[b].T)
        in_maps.append(
            dict(xt=xt, wq=wq, wk=wk, wv=wv, wp=wp, bq=bq, bk=bk,
                 bvr=np.ascontiguousarray(bvr), rope=rope, tri=tri)
        )
    return in_maps


def kernel(x, W_qkv, b_qkv, W_proj, b_proj):
    from concourse.bass_utils import run_bass_kernel_spmd
    import concourse.bass_utils as bass_utils

    x = np.asarray(x, dtype=np.float32)
    W_qkv = np.asarray(W_qkv, dtype=np.float32)
    b_qkv = np.asarray(b_qkv, dtype=np.float32)
    W_proj = np.asarray(W_proj, dtype=np.float32)
    b_proj = np.asarray(b_proj, dtype=np.float32)

    trace = bool(os.environ.get("BASS_KERNEL_TRACE"))
    if trace:
        _install_ntff_shim()
        bass_utils.upload_artifacts = lambda tmpdir: "local://" + tmpdir

    nc = _build()
    in_maps = _host_prep(x, W_qkv, b_qkv, W_proj)
    kw = {}
    if trace:
        tdir = os.environ.get("BASS_KERNEL_TRACE_DIR", "/tmp/ksa_trace")
        os.makedirs(tdir, exist_ok=True)
        kw = dict(trace=True, tmpdir=tdir)
    try:
        res = run_bass_kernel_spmd(nc, in_maps, core_ids=list(range(NC)), **kw)
    except Exception:
        # transient NRT_EXEC_UNIT_UNRECOVERABLE has been observed on the
        # first execution after a fresh compile; one retry clears it
        res = run_bass_kernel_spmd(nc, in_maps, core_ids=list(range(NC)), **kw)
    _BUILD_CACHE["last_result"] = res

    out = np.empty((B, S, C), np.float32)
    for b in range(B):
        oT = res.results[2 * b]["o"] + res.results[2 * b + 1]["o"]
        out[b] = oT.T + b_proj[None, :]
    return out


# revision 35
# speedup vs baseline: 1.3202x; 1.0496x over previous
"""Causal self-attention (RoPE) Trainium2 kernel, 8-core SPMD.

Sharding: core = (batch b, head-group g) -> 4 batches x 2 groups of 8 heads.
Each core: QKV projection for its 8 heads, RoPE, causal attention
(lazy softmax: unnormalized att @ [V|1] gives numerator + row-sums in one
matmul), then a partial output projection o^T = Wp_g^T @ y^T.
Host sums the two partial o^T per batch and adds b_proj.

Matmuls: float32r (fp32-data fast PE mode) except scores, which run in
bf16 on pair-layout copies (kt2/qt2) so each head's 64 dims sit on 64
contiguous partitions -> one K=64 matmul per score block, two heads
row-packed via tile_position.

Layouts (per core):
  xt  = x[b]^T [1024, 2048];  wq/wk [1024, 512] pi-permuted columns
  (chunks = [E h0-3, E h4-7, O h0-3, O h4-7], E/O = even/odd head dims) so
  RoPE pair-rotation is full-width [128, n] DVE ops across chunk pairs.
  wv [1024, 512] natural;  wp [512, 1024] rows = this core's head dims.
  kt2/qt2 (rotated, bf16) chunk m = heads (2m, 2m+1) x [E32|O32] each.
  v_sb [S, 520] f32r (65 cols/head: 64 v + ones col for the row-sum trick)
  scores transposed s^T[k, q]: out^T[d, q] = [V|1]^T @ exp(s^T), row 64 =
  softmax denominator.  Diagonal score blocks are N-trimmed to the causal
  range; one [128,128] lower-triangle mask handles the block diagonal.
"""

import os
import sys

sys.path.insert(0, "/opt/trn_rl_repo")

import numpy as np
import ml_dtypes

B, S, C = 4, 2048, 1024
H, D = 16, 64
NC = 8
ROPE_BASE = 10000.0
SQ = 512          # q-chunk / streaming chunk
NJ = S // SQ      # 4

_BUILD_CACHE = {}


def _install_ntff_shim():
    """Optional: register antenv.axon_hooks so trace=True works under axon."""
    import contextlib
    import ctypes
    import types

    if "antenv.axon_hooks" in sys.modules:
        return
    so_path = "/opt/axon/libaxon_pjrt.so"
    if not os.path.exists(so_path):
        return
    lib = ctypes.CDLL(so_path)
    if not hasattr(lib, "axon_start_nrt_profile"):
        return
    lib.axon_start_nrt_profile.argtypes = [ctypes.POINTER(ctypes.c_int64), ctypes.c_size_t]
    lib.axon_start_nrt_profile.restype = ctypes.c_int64
    lib.axon_stop_nrt_profile.argtypes = [ctypes.c_char_p]
    lib.axon_stop_nrt_profile.restype = ctypes.c_int64

    @contextlib.contextmanager
    def _hook(output_dir, device_ids):
        import jax

        jax.devices()
        if device_ids:
            ids = (ctypes.c_int64 * len(device_ids))(*device_ids)
            rc = lib.axon_start_nrt_profile(ids, len(device_ids))
        else:
            rc = lib.axon_start_nrt_profile(None, 0)
        if rc != 0:
            raise RuntimeError(f"axon_start_nrt_profile rc={rc}")
        try:
            yield
        finally:
            lib.axon_stop_nrt_profile(output_dir.encode())

    mod = types.ModuleType("antenv.axon_hooks")
    mod.get_axon_ntff_profile_hook = lambda: _hook
    mod.set_axon_ntff_profile_hook = lambda h: None
    sys.modules["antenv.axon_hooks"] = mod


def _build():
    if "nc" in _BUILD_CACHE:
        return _BUILD_CACHE["nc"]
    import concourse.bacc as bacc
    import concourse.mybir as mybir
    from concourse.tile import TileContext

    f32 = mybir.dt.float32
    f32r = mybir.dt.float32r
    bf16 = mybir.dt.bfloat16
    ALU = mybir.AluOpType
    ACTF = mybir.ActivationFunctionType

    nc = bacc.Bacc("TRN2", target_bir_lowering=False, debug=False, num_devices=NC)

    xt_d = nc.dram_tensor("xt", [C, S], bf16, kind="ExternalInput")
    wq_d = nc.dram_tensor("wq", [C, 512], bf16, kind="ExternalInput")
    wk_d = nc.dram_tensor("wk", [C, 512], bf16, kind="ExternalInput")
    wv_d = nc.dram_tensor("wv", [C, 512], bf16, kind="ExternalInput")
    wp_d = nc.dram_tensor("wp", [512, C], f32r, kind="ExternalInput")
    bq_d = nc.dram_tensor("bq", [128, 4], f32, kind="ExternalInput")
    bk_d = nc.dram_tensor("bk", [128, 4], f32, kind="ExternalInput")
    bvr_d = nc.dram_tensor("bvr", [128, 512], f32, kind="ExternalInput")
    rope_d = nc.dram_tensor("rope", [4, 128, S], f32, kind="ExternalInput")
    tri_d = nc.dram_tensor("tri", [128, 128], bf16, kind="ExternalInput")
    o_d = nc.dram_tensor("o", [C, S], f32, kind="ExternalOutput")
    dbg = bool(os.environ.get("KSA_DEBUG"))
    if dbg:
        kt2_dbg = nc.dram_tensor("kt2_dbg", [128, 4, S], bf16, kind="ExternalOutput")
        v_dbg = nc.dram_tensor("v_dbg", [128, S // 128, 520], f32r, kind="ExternalOutput")
        qt2_dbg = nc.dram_tensor("qt2_dbg", [128, 4, SQ], bf16, kind="ExternalOutput")
        att_dbg = nc.dram_tensor("att_dbg", [128, 2, SQ], f32r, kind="ExternalOutput")
        av_dbg = nc.dram_tensor("av_dbg", [65, SQ], f32, kind="ExternalOutput")
        rec_dbg = nc.dram_tensor("rec_dbg", [64, SQ], f32, kind="ExternalOutput")
        yt_dbg = nc.dram_tensor("yt_dbg", [128, 4, SQ], f32r, kind="ExternalOutput")

    with TileContext(nc) as tc, tc.tile_pool(name="const", bufs=1) as constp:
        wq_sb = constp.tile([128, 8, 512], bf16, name="wq_sb", tag="wq")
        wk_sb = constp.tile([128, 8, 512], bf16, name="wk_sb", tag="wk")
        wv_sb = constp.tile([128, 8, 512], bf16, name="wv_sb", tag="wv")
        wp_sb = constp.tile([128, 4, C], f32r, name="wp_sb", tag="wp")
        bq_sb = constp.tile([128, 4], f32, name="bq_sb", tag="bq")
        bk_sb = constp.tile([128, 4], f32, name="bk_sb", tag="bk")
        bvr_sb = constp.tile([128, 512], f32, name="bvr_sb", tag="bvr")
        tri_sb = constp.tile([128, 128], bf16, name="tri_sb", tag="tri")
        ones1 = constp.tile([1, 64], f32r, name="ones1", tag="ones1")
        kt2 = constp.tile([128, 4, S], bf16, name="kt2", tag="kt2")
        v_sb = constp.tile([128, S // 128, 520], bf16, name="v_sb", tag="v")

        nc.sync.dma_start(out=wq_sb[:], in_=wq_d.rearrange("(c p) d -> p c d", p=128))
        nc.sync.dma_start(out=wk_sb[:], in_=wk_d.rearrange("(c p) d -> p c d", p=128))
        nc.sync.dma_start(out=wv_sb[:], in_=wv_d.rearrange("(c p) d -> p c d", p=128))
        nc.sync.dma_start(out=wp_sb[:], in_=wp_d.rearrange("(c p) o -> p c o", p=128))
        nc.sync.dma_start(out=bq_sb[:], in_=bq_d[:])
        nc.sync.dma_start(out=bk_sb[:], in_=bk_d[:])
        nc.sync.dma_start(out=bvr_sb[:], in_=bvr_d[:])
        nc.sync.dma_start(out=tri_sb[:], in_=tri_d[:])
        nc.vector.memset(ones1[:].bitcast(f32), 1.0)
        for hh in range(8):
            nc.vector.memset(v_sb[:, :, 65 * hh + 64 : 65 * hh + 65], 1.0)

        with (
            tc.tile_pool(name="xtp", bufs=2) as xtp,
            tc.tile_pool(name="ropep", bufs=2) as ropep,
            tc.tile_pool(name="scrp", bufs=6) as scrp,
            tc.tile_pool(name="eop", bufs=3) as eop,
            tc.tile_pool(name="qt2p", bufs=2) as qt2p,
            tc.tile_pool(name="attp", bufs=3) as attp,
            tc.tile_pool(name="yp", bufs=1) as yp,
            tc.tile_pool(name="otp", bufs=2) as otp,
            tc.tile_pool(name="psp", bufs=3, space="PSUM") as psp,
            tc.tile_pool(name="avp", bufs=2, space="PSUM") as avp,
        ):
            for j in range(NJ):
                s0 = j * SQ
                # ---- phase A: QKV + RoPE --------------------------------
                xt_sb = xtp.tile([128, 8, SQ], bf16, tag="xt")
                nc.sync.dma_start(
                    out=xt_sb[:],
                    in_=xt_d.rearrange("(c p) s -> p c s", p=128)[:, :, s0 : s0 + SQ],
                )
                rp = ropep.tile([128, 4, SQ], f32, tag="rope")
                nc.sync.dma_start(
                    out=rp[:],
                    in_=rope_d.rearrange("t p s -> p t s")[:, :, s0 : s0 + SQ],
                )
                qt_eo = eop.tile([128, 4, SQ], bf16, tag="eo", name="qt_eo")
                kt_eo = eop.tile([128, 4, SQ], bf16, tag="eo", name="kt_eo")
                # cc pairs: psum tile holds (E chunk cc, O chunk cc)
                for (src_w, bias, eo_t) in (
                    (wq_sb, bq_sb, qt_eo),
                    (wk_sb, bk_sb, kt_eo),
                ):
                    for cc in range(2):
                        pch = [cc, 2 + cc]  # pi chunk indices (E_cc, O_cc)
                        ps = psp.tile([128, 2, SQ], f32, tag="ps", name="qk_ps")
                        for sl in range(2):
                            dcol = 128 * pch[sl]
                            for c in range(8):
                                nc.tensor.matmul(
                                    ps[:, sl, :],
                                    src_w[:, c, dcol : dcol + 128],
                                    xt_sb[:, c, :],
                                    start=(c == 0),
                                    stop=(c == 7),
                                )
                        # rotE = (E+b)ce - (O+b)se ; rotO = (O+b)co + (E+b)so
                        t1 = scrp.tile([128, SQ], f32, tag="scr")
                        t2 = scrp.tile([128, SQ], f32, tag="scr")
                        t3 = scrp.tile([128, SQ], f32, tag="scr")
                        t4 = scrp.tile([128, SQ], f32, tag="scr")
                        nc.vector.scalar_tensor_tensor(
                            t1[:], ps[:, 0, :], bias[:, pch[0] : pch[0] + 1],
                            rp[:, 0, :], ALU.add, ALU.mult,
                        )
                        nc.vector.scalar_tensor_tensor(
                            t2[:], ps[:, 1, :], bias[:, pch[1] : pch[1] + 1],
                            rp[:, 1, :], ALU.add, ALU.mult,
                        )
                        nc.vector.scalar_tensor_tensor(
                            t3[:], ps[:, 1, :], bias[:, pch[1] : pch[1] + 1],
                            rp[:, 2, :], ALU.add, ALU.mult,
                        )
                        nc.vector.scalar_tensor_tensor(
                            t4[:], ps[:, 0, :], bias[:, pch[0] : pch[0] + 1],
                            rp[:, 3, :], ALU.add, ALU.mult,
                        )
                        nc.gpsimd.tensor_sub(eo_t[:, pch[0], :], t1[:], t2[:])
                        nc.gpsimd.tensor_add(eo_t[:, pch[1], :], t3[:], t4[:])
                # v projection (natural layout, [s,d], 128-row subchunks)
                for uu in range(2):
                    v_ps = psp.tile([128, 2, SQ], f32, tag="ps", name="v_ps")
                    for u in range(2):
                        for c in range(8):
                            nc.tensor.matmul(
                                v_ps[:, u, :],
                                xt_sb[:, c, 256 * uu + 128 * u : 256 * uu + 128 * u + 128],
                                wv_sb[:, c, :],
                                start=(c == 0),
                                stop=(c == 7),
                            )
                    for u in range(2):
                        ch = 4 * j + 2 * uu + u
                        dst = v_sb[:, ch].rearrange("p (h d) -> p h d", d=65)[:, :, 0:64]
                        nc.vector.tensor_tensor(dst, v_ps[:, u, :], bvr_sb[:], ALU.add)
                # pair-layout copies (E/O chunks -> per-head 64-row bands)
                qt2 = qt2p.tile([128, 4, SQ], bf16, tag="qt2")
                for hh in range(8):
                    se_p, se_c = 32 * (hh % 4), hh // 4
                    de_p, de_c = 64 * (hh % 2), hh // 2
                    nc.sync.dma_start(
                        out=qt2[de_p : de_p + 32, de_c, :],
                        in_=qt_eo[se_p : se_p + 32, se_c, :],
                    )
                    nc.sync.dma_start(
                        out=qt2[de_p + 32 : de_p + 64, de_c, :],
                        in_=qt_eo[se_p : se_p + 32, 2 + se_c, :],
                    )
                    nc.sync.dma_start(
                        out=kt2[de_p : de_p + 32, de_c, s0 : s0 + SQ],
                        in_=kt_eo[se_p : se_p + 32, se_c, :],
                    )
                    nc.sync.dma_start(
                        out=kt2[de_p + 32 : de_p + 64, de_c, s0 : s0 + SQ],
                        in_=kt_eo[se_p : se_p + 32, 2 + se_c, :],
                    )

                if dbg and j == 0:
                    nc.sync.dma_start(out=qt2_dbg[:], in_=qt2[:])
                # ---- phase B: attention ---------------------------------
                nkb = 4 * j + 4
                yt = yp.tile([128, 4, SQ], f32r, tag="yt")
                for p in range(4):
                    avs = [
                        avp.tile([65, SQ], f32, tag="av", name=f"av{j}_{p}_{t}")
                        for t in range(2)
                    ]
                    for c in range(nkb):
                        off = 128 * (c - 4 * j) if c >= 4 * j else 0
                        sc = psp.tile([128, 2, SQ], f32, tag="ps", name="sc")
                        for t in range(2):
                            hi = 2 * p + t
                            rb, ch = 64 * (hi % 2), hi // 2
                            nc.tensor.matmul(
                                sc[:, t, off:SQ],
                                kt2[rb : rb + 64, ch, 128 * c : 128 * c + 128],
                                qt2[rb : rb + 64, ch, off:SQ],
                                start=True,
                                stop=True,
                                tile_position=(rb, 0),
                            )
                        att = attp.tile([128, 2, SQ], bf16, tag="att")
                        nc.scalar.activation(
                            att[:, :, off:SQ], sc[:, :, off:SQ], ACTF.Exp, scale=0.125
                        )
                        if c >= 4 * j:
                            for t in range(2):
                                nc.gpsimd.tensor_tensor(
                                    att[:, t, off : off + 128],
                                    att[:, t, off : off + 128],
                                    tri_sb[:],
                                    ALU.mult,
                                )
                        if dbg and j == 0 and p == 0 and c == 0:
                            nc.sync.dma_start(out=att_dbg[:], in_=att[:])
                        for t in range(2):
                            hi = 2 * p + t
                            nc.tensor.matmul(
                                avs[t][0:65, off:SQ],
                                v_sb[:, c, 65 * hi : 65 * hi + 65],
                                att[:, t, off:SQ],
                                start=(c == 0),
                                stop=(c == nkb - 1),
                                skip_group_check=True,
                            )
                    if dbg and j == 0 and p == 0:
                        av_cp = otp.tile([65, SQ], f32, tag="ot", name="av_cp")
                        nc.vector.tensor_copy(av_cp[:], avs[0][:])
                        nc.sync.dma_start(out=av_dbg[:], in_=av_cp[:])
                    for t in range(2):
                        hi = 2 * p + t
                        yu = scrp.tile([65, SQ], f32, tag="scr", name="yu")
                        den0 = scrp.tile([1, SQ], f32, tag="scr", name="den0")
                        rec_f = scrp.tile([1, SQ], f32, tag="scr", name="rec_f")
                        rec_r = scrp.tile([1, SQ], f32r, tag="scr", name="rec_r")
                        nc.scalar.copy(yu[:], avs[t][:])
                        nc.vector.tensor_copy(den0[:], avs[t][64:65, :])
                        nc.vector.reciprocal_approx_fast(rec_f[:], den0[:])
                        nc.vector.tensor_copy(rec_r[:], rec_f[:])
                        bc = avp.tile([64, SQ], f32, tag="av", name="bc")
                        nc.tensor.matmul(bc[:], ones1[:], rec_r[:], start=True, stop=True)
                        nc.vector.tensor_mul(
                            yt[64 * (hi % 2) : 64 * (hi % 2) + 64, hi // 2, :],
                            yu[0:64, :],
                            bc[:],
                        )
                        if dbg and j == 0 and p == 0 and t == 0:
                            nc.sync.dma_start(out=rec_dbg[:], in_=rec64[:])

                if dbg and j == 0:
                    nc.sync.dma_start(out=yt_dbg[:], in_=yt[:])
                # ---- phase C: partial projection ------------------------
                for co in range(8):
                    pj = psp.tile([128, SQ], f32, tag="ps", name="pj")
                    for cin in range(4):
                        nc.tensor.matmul(
                            pj[:],
                            wp_sb[:, cin, 128 * co : 128 * co + 128],
                            yt[:, cin, :],
                            start=(cin == 0),
                            stop=(cin == 3),
                        )
                    ot = otp.tile([128, SQ], f32, tag="ot")
                    nc.vector.tensor_copy(ot[:], pj[:])
                    nc.sync.dma_start(
                        out=o_d[128 * co : 128 * co + 128, s0 : s0 + SQ],
                        in_=ot[:],
                    )

        if dbg:
            nc.sync.dma_start(out=kt2_dbg[:], in_=kt2[:])
            nc.sync.dma_start(out=v_dbg[:], in_=v_sb[:])

    nc.compile()
    _BUILD_CACHE["nc"] = nc
    return nc


def _host_prep(x, W_qkv, b_qkv, W_proj):
    """Build the 8 per-core input maps."""
    inv_freq = (1.0 / (ROPE_BASE ** (np.arange(0, D, 2, dtype=np.float32) / D))).astype(
        np.float32
    )
    pos = np.arange(S, dtype=np.float32)
    freqs = np.outer(pos, inv_freq).astype(np.float32)  # [S, 32]
    emb = np.concatenate([freqs, freqs], axis=-1)  # [S, 64]
    cos_t = np.cos(emb).astype(np.float32)
    sin_t = np.sin(emb).astype(np.float32)
    m = np.arange(32)
    ce = np.ascontiguousarray(cos_t[:, 2 * m].T)  # [32, S]
    se = np.ascontiguousarray(sin_t[:, 2 * m].T)
    co = np.ascontiguousarray(cos_t[:, 2 * m + 1].T)
    so = np.ascontiguousarray(sin_t[:, 2 * m + 1].T)
    rope = np.stack([np.tile(t, (4, 1)) for t in (ce, se, co, so)], axis=0)

    qq = np.arange(128)[None, :]
    rr = np.arange(128)[:, None]
    tri = (rr <= qq).astype(ml_dtypes.bfloat16)

    in_maps = []
    for core in range(NC):
        b, g = core // 2, core % 2
        heads = list(range(8 * g, 8 * g + 8))
        cols_e = [64 * heads[4 * c4 + i] + 2 * mm
                  for c4 in range(2) for i in range(4) for mm in range(32)]
        cols_o = [64 * heads[4 * c4 + i] + 2 * mm + 1
                  for c4 in range(2) for i in range(4) for mm in range(32)]
        pi = np.array(cols_e + cols_o)
        vcols = np.array([64 * h + d for h in heads for d in range(64)])

        wq = np.ascontiguousarray(W_qkv[:, pi]).astype(ml_dtypes.bfloat16)
        wk = np.ascontiguousarray(W_qkv[:, C + pi]).astype(ml_dtypes.bfloat16)
        wv = np.ascontiguousarray(W_qkv[:, 2 * C + vcols]).astype(ml_dtypes.bfloat16)
        wp = np.ascontiguousarray(W_proj[vcols, :])
        bq = np.ascontiguousarray(b_qkv[pi].reshape(4, 128).T)
        bk = np.ascontiguousarray(b_qkv[C + pi].reshape(4, 128).T)
        bvr = np.tile(b_qkv[2 * C + vcols][None, :], (128, 1))
        xt = np.ascontiguousarray(x[b].T).astype(ml_dtypes.bfloat16)
        in_maps.append(
            dict(xt=xt, wq=wq, wk=wk, wv=wv, wp=wp, bq=bq, bk=bk,
                 bvr=np.ascontiguousarray(bvr), rope=rope, tri=tri)
        )
    return in_maps


def kernel(x, W_qkv, b_qkv, W_proj, b_proj):
    from concourse.bass_utils import run_bass_kernel_spmd
    import concourse.bass_utils as bass_utils

    x = np.asarray(x, dtype=np.float32)
    W_qkv = np.asarray(W_qkv, dtype=np.float32)
    b_qkv = np.asarray(b_qkv, dtype=np.float32)
    W_proj = np.asarray(W_proj, dtype=np.float32)
    b_proj = np.asarray(b_proj, dtype=np.float32)

    trace = bool(os.environ.get("BASS_KERNEL_TRACE"))
    if trace:
        _install_ntff_shim()
        bass_utils.upload_artifacts = lambda tmpdir: "local://" + tmpdir

    nc = _build()
    in_maps = _host_prep(x, W_qkv, b_qkv, W_proj)
    kw = {}
    if trace:
        tdir = os.environ.get("BASS_KERNEL_TRACE_DIR", "/tmp/ksa_trace")
        os.makedirs(tdir, exist_ok=True)
        kw = dict(trace=True, tmpdir=tdir)
    try:
        res = run_bass_kernel_spmd(nc, in_maps, core_ids=list(range(NC)), **kw)
    except Exception:
        # transient NRT_EXEC_UNIT_UNRECOVERABLE has been observed on the
        # first execution after a fresh compile; one retry clears it
        res = run_bass_kernel_spmd(nc, in_maps, core_ids=list(range(NC)), **kw)
    _BUILD_CACHE["last_result"] = res

    out = np.empty((B, S, C), np.float32)
    for b in range(B):
        oT = res.results[2 * b]["o"] + res.results[2 * b + 1]["o"]
        out[b] = oT.T + b_proj[None, :]
    return out


# revision 36
# speedup vs baseline: 1.3799x; 1.0452x over previous
"""Causal self-attention (RoPE) Trainium2 kernel, 8-core SPMD.

Sharding: core = (batch b, head-group g) -> 4 batches x 2 groups of 8 heads.
Each core: QKV projection for its 8 heads, RoPE, causal attention
(lazy softmax: unnormalized att @ [V|1] gives numerator + row-sums in one
matmul), then a partial output projection o^T = Wp_g^T @ y^T.
Host sums the two partial o^T per batch and adds b_proj.

Matmuls: float32r (fp32-data fast PE mode) except scores, which run in
bf16 on pair-layout copies (kt2/qt2) so each head's 64 dims sit on 64
contiguous partitions -> one K=64 matmul per score block, two heads
row-packed via tile_position.

Layouts (per core):
  xt  = x[b]^T [1024, 2048];  wq/wk [1024, 512] pi-permuted columns
  (chunks = [E h0-3, E h4-7, O h0-3, O h4-7], E/O = even/odd head dims) so
  RoPE pair-rotation is full-width [128, n] DVE ops across chunk pairs.
  wv [1024, 512] natural;  wp [512, 1024] rows = this core's head dims.
  kt2/qt2 (rotated, bf16) chunk m = heads (2m, 2m+1) x [E32|O32] each.
  v_sb [S, 520] f32r (65 cols/head: 64 v + ones col for the row-sum trick)
  scores transposed s^T[k, q]: out^T[d, q] = [V|1]^T @ exp(s^T), row 64 =
  softmax denominator.  Diagonal score blocks are N-trimmed to the causal
  range; one [128,128] lower-triangle mask handles the block diagonal.
"""

import os
import sys

sys.path.insert(0, "/opt/trn_rl_repo")

import numpy as np
import ml_dtypes

B, S, C = 4, 2048, 1024
H, D = 16, 64
NC = 8
ROPE_BASE = 10000.0
SQ = 512          # q-chunk / streaming chunk
NJ = S // SQ      # 4

_BUILD_CACHE = {}


def _install_ntff_shim():
    """Optional: register antenv.axon_hooks so trace=True works under axon."""
    import contextlib
    import ctypes
    import types

    if "antenv.axon_hooks" in sys.modules:
        return
    so_path = "/opt/axon/libaxon_pjrt.so"
    if not os.path.exists(so_path):
        return
    lib = ctypes.CDLL(so_path)
    if not hasattr(lib, "axon_start_nrt_profile"):
        return
    lib.axon_start_nrt_profile.argtypes = [ctypes.POINTER(ctypes.c_int64), ctypes.c_size_t]
    lib.axon_start_nrt_profile.restype = ctypes.c_int64
    lib.axon_stop_nrt_profile.argtypes = [ctypes.c_char_p]
    lib.axon_stop_nrt_profile.restype = ctypes.c_int64

    @contextlib.contextmanager
    def _hook(output_dir, device_ids):
        import jax

        jax.devices()
        if device_ids:
            ids = (ctypes.c_int64 * len(device_ids))(*device_ids)
            rc = lib.axon_start_nrt_profile(ids, len(device_ids))
        else:
            rc = lib.axon_start_nrt_profile(None, 0)
        if rc != 0:
            raise RuntimeError(f"axon_start_nrt_profile rc={rc}")
        try:
            yield
        finally:
            lib.axon_stop_nrt_profile(output_dir.encode())

    mod = types.ModuleType("antenv.axon_hooks")
    mod.get_axon_ntff_profile_hook = lambda: _hook
    mod.set_axon_ntff_profile_hook = lambda h: None
    sys.modules["antenv.axon_hooks"] = mod


def _build():
    if "nc" in _BUILD_CACHE:
        return _BUILD_CACHE["nc"]
    import concourse.bacc as bacc
    import concourse.mybir as mybir
    from concourse.tile import TileContext

    f32 = mybir.dt.float32
    f32r = mybir.dt.float32r
    bf16 = mybir.dt.bfloat16
    ALU = mybir.AluOpType
    ACTF = mybir.ActivationFunctionType

    nc = bacc.Bacc("TRN2", target_bir_lowering=False, debug=False, num_devices=NC)

    xt_d = nc.dram_tensor("xt", [C, S], bf16, kind="ExternalInput")
    wq_d = nc.dram_tensor("wq", [C, 512], bf16, kind="ExternalInput")
    wk_d = nc.dram_tensor("wk", [C, 512], bf16, kind="ExternalInput")
    wv_d = nc.dram_tensor("wv", [C, 512], bf16, kind="ExternalInput")
    wp_d = nc.dram_tensor("wp", [512, C], bf16, kind="ExternalInput")
    bq_d = nc.dram_tensor("bq", [128, 4], f32, kind="ExternalInput")
    bk_d = nc.dram_tensor("bk", [128, 4], f32, kind="ExternalInput")
    bvr_d = nc.dram_tensor("bvr", [128, 512], f32, kind="ExternalInput")
    rope_d = nc.dram_tensor("rope", [4, 128, S], f32, kind="ExternalInput")
    tri_d = nc.dram_tensor("tri", [128, 128], bf16, kind="ExternalInput")
    o_d = nc.dram_tensor("o", [C, S], f32, kind="ExternalOutput")
    dbg = bool(os.environ.get("KSA_DEBUG"))
    if dbg:
        kt2_dbg = nc.dram_tensor("kt2_dbg", [128, 4, S], bf16, kind="ExternalOutput")
        v_dbg = nc.dram_tensor("v_dbg", [128, S // 128, 520], f32r, kind="ExternalOutput")
        qt2_dbg = nc.dram_tensor("qt2_dbg", [128, 4, SQ], bf16, kind="ExternalOutput")
        att_dbg = nc.dram_tensor("att_dbg", [128, 2, SQ], f32r, kind="ExternalOutput")
        av_dbg = nc.dram_tensor("av_dbg", [65, SQ], f32, kind="ExternalOutput")
        rec_dbg = nc.dram_tensor("rec_dbg", [64, SQ], f32, kind="ExternalOutput")
        yt_dbg = nc.dram_tensor("yt_dbg", [128, 4, SQ], f32r, kind="ExternalOutput")

    with TileContext(nc) as tc, tc.tile_pool(name="const", bufs=1) as constp:
        wq_sb = constp.tile([128, 8, 512], bf16, name="wq_sb", tag="wq")
        wk_sb = constp.tile([128, 8, 512], bf16, name="wk_sb", tag="wk")
        wv_sb = constp.tile([128, 8, 512], bf16, name="wv_sb", tag="wv")
        wp_sb = constp.tile([128, 4, C], bf16, name="wp_sb", tag="wp")
        bq_sb = constp.tile([128, 4], f32, name="bq_sb", tag="bq")
        bk_sb = constp.tile([128, 4], f32, name="bk_sb", tag="bk")
        bvr_sb = constp.tile([128, 512], f32, name="bvr_sb", tag="bvr")
        tri_sb = constp.tile([128, 128], bf16, name="tri_sb", tag="tri")
        ones1 = constp.tile([1, 64], f32r, name="ones1", tag="ones1")
        kt2 = constp.tile([128, 4, S], bf16, name="kt2", tag="kt2")
        v_sb = constp.tile([128, S // 128, 520], bf16, name="v_sb", tag="v")

        nc.sync.dma_start(out=wq_sb[:], in_=wq_d.rearrange("(c p) d -> p c d", p=128))
        nc.sync.dma_start(out=wk_sb[:], in_=wk_d.rearrange("(c p) d -> p c d", p=128))
        nc.sync.dma_start(out=wv_sb[:], in_=wv_d.rearrange("(c p) d -> p c d", p=128))
        nc.sync.dma_start(out=wp_sb[:], in_=wp_d.rearrange("(c p) o -> p c o", p=128))
        nc.sync.dma_start(out=bq_sb[:], in_=bq_d[:])
        nc.sync.dma_start(out=bk_sb[:], in_=bk_d[:])
        nc.sync.dma_start(out=bvr_sb[:], in_=bvr_d[:])
        nc.sync.dma_start(out=tri_sb[:], in_=tri_d[:])
        nc.vector.memset(ones1[:].bitcast(f32), 1.0)
        for hh in range(8):
            nc.vector.memset(v_sb[:, :, 65 * hh + 64 : 65 * hh + 65], 1.0)

        with (
            tc.tile_pool(name="xtp", bufs=2) as xtp,
            tc.tile_pool(name="ropep", bufs=2) as ropep,
            tc.tile_pool(name="scrp", bufs=8) as scrp,
            tc.tile_pool(name="eop", bufs=3) as eop,
            tc.tile_pool(name="qt2p", bufs=2) as qt2p,
            tc.tile_pool(name="attp", bufs=4) as attp,
            tc.tile_pool(name="yp", bufs=1) as yp,
            tc.tile_pool(name="otp", bufs=2) as otp,
            tc.tile_pool(name="psp", bufs=3, space="PSUM") as psp,
            tc.tile_pool(name="avp", bufs=2, space="PSUM") as avp,
        ):
            for j in range(NJ):
                s0 = j * SQ
                # ---- phase A: QKV + RoPE --------------------------------
                xt_sb = xtp.tile([128, 8, SQ], bf16, tag="xt")
                nc.sync.dma_start(
                    out=xt_sb[:],
                    in_=xt_d.rearrange("(c p) s -> p c s", p=128)[:, :, s0 : s0 + SQ],
                )
                rp = ropep.tile([128, 4, SQ], f32, tag="rope")
                nc.sync.dma_start(
                    out=rp[:],
                    in_=rope_d.rearrange("t p s -> p t s")[:, :, s0 : s0 + SQ],
                )
                qt_eo = eop.tile([128, 4, SQ], bf16, tag="eo", name="qt_eo")
                kt_eo = eop.tile([128, 4, SQ], bf16, tag="eo", name="kt_eo")
                # cc pairs: psum tile holds (E chunk cc, O chunk cc)
                for (src_w, bias, eo_t) in (
                    (wq_sb, bq_sb, qt_eo),
                    (wk_sb, bk_sb, kt_eo),
                ):
                    for cc in range(2):
                        pch = [cc, 2 + cc]  # pi chunk indices (E_cc, O_cc)
                        ps = psp.tile([128, 2, SQ], f32, tag="ps", name="qk_ps")
                        for sl in range(2):
                            dcol = 128 * pch[sl]
                            for c in range(8):
                                nc.tensor.matmul(
                                    ps[:, sl, :],
                                    src_w[:, c, dcol : dcol + 128],
                                    xt_sb[:, c, :],
                                    start=(c == 0),
                                    stop=(c == 7),
                                )
                        # rotE = (E+b)ce - (O+b)se ; rotO = (O+b)co + (E+b)so
                        t1 = scrp.tile([128, SQ], f32, tag="scr")
                        t2 = scrp.tile([128, SQ], f32, tag="scr")
                        t3 = scrp.tile([128, SQ], f32, tag="scr")
                        t4 = scrp.tile([128, SQ], f32, tag="scr")
                        nc.vector.scalar_tensor_tensor(
                            t1[:], ps[:, 0, :], bias[:, pch[0] : pch[0] + 1],
                            rp[:, 0, :], ALU.add, ALU.mult,
                        )
                        nc.vector.scalar_tensor_tensor(
                            t2[:], ps[:, 1, :], bias[:, pch[1] : pch[1] + 1],
                            rp[:, 1, :], ALU.add, ALU.mult,
                        )
                        nc.vector.scalar_tensor_tensor(
                            t3[:], ps[:, 1, :], bias[:, pch[1] : pch[1] + 1],
                            rp[:, 2, :], ALU.add, ALU.mult,
                        )
                        nc.vector.scalar_tensor_tensor(
                            t4[:], ps[:, 0, :], bias[:, pch[0] : pch[0] + 1],
                            rp[:, 3, :], ALU.add, ALU.mult,
                        )
                        nc.gpsimd.tensor_sub(eo_t[:, pch[0], :], t1[:], t2[:])
                        nc.gpsimd.tensor_add(eo_t[:, pch[1], :], t3[:], t4[:])
                # v projection (natural layout, [s,d], 128-row subchunks)
                for uu in range(2):
                    v_ps = psp.tile([128, 2, SQ], f32, tag="ps", name="v_ps")
                    for u in range(2):
                        for c in range(8):
                            nc.tensor.matmul(
                                v_ps[:, u, :],
                                xt_sb[:, c, 256 * uu + 128 * u : 256 * uu + 128 * u + 128],
                                wv_sb[:, c, :],
                                start=(c == 0),
                                stop=(c == 7),
                            )
                    for u in range(2):
                        ch = 4 * j + 2 * uu + u
                        dst = v_sb[:, ch].rearrange("p (h d) -> p h d", d=65)[:, :, 0:64]
                        nc.vector.tensor_tensor(dst, v_ps[:, u, :], bvr_sb[:], ALU.add)
                # pair-layout copies (E/O chunks -> per-head 64-row bands)
                qt2 = qt2p.tile([128, 4, SQ], bf16, tag="qt2")
                for hh in range(8):
                    se_p, se_c = 32 * (hh % 4), hh // 4
                    de_p, de_c = 64 * (hh % 2), hh // 2
                    nc.sync.dma_start(
                        out=qt2[de_p : de_p + 32, de_c, :],
                        in_=qt_eo[se_p : se_p + 32, se_c, :],
                    )
                    nc.sync.dma_start(
                        out=qt2[de_p + 32 : de_p + 64, de_c, :],
                        in_=qt_eo[se_p : se_p + 32, 2 + se_c, :],
                    )
                    nc.sync.dma_start(
                        out=kt2[de_p : de_p + 32, de_c, s0 : s0 + SQ],
                        in_=kt_eo[se_p : se_p + 32, se_c, :],
                    )
                    nc.sync.dma_start(
                        out=kt2[de_p + 32 : de_p + 64, de_c, s0 : s0 + SQ],
                        in_=kt_eo[se_p : se_p + 32, 2 + se_c, :],
                    )

                if dbg and j == 0:
                    nc.sync.dma_start(out=qt2_dbg[:], in_=qt2[:])
                # ---- phase B: attention ---------------------------------
                nkb = 4 * j + 4
                yt = yp.tile([128, 4, SQ], bf16, tag="yt")
                for p in range(4):
                    avs = [
                        avp.tile([65, SQ], f32, tag="av", name=f"av{j}_{p}_{t}")
                        for t in range(2)
                    ]
                    for c in range(nkb):
                        off = 128 * (c - 4 * j) if c >= 4 * j else 0
                        sc = psp.tile([128, 2, SQ], f32, tag="ps", name="sc")
                        for t in range(2):
                            hi = 2 * p + t
                            rb, ch = 64 * (hi % 2), hi // 2
                            nc.tensor.matmul(
                                sc[:, t, off:SQ],
                                kt2[rb : rb + 64, ch, 128 * c : 128 * c + 128],
                                qt2[rb : rb + 64, ch, off:SQ],
                                start=True,
                                stop=True,
                                tile_position=(rb, 0),
                            )
                        att = attp.tile([128, 2, SQ], bf16, tag="att")
                        nc.scalar.activation(
                            att[:, :, off:SQ], sc[:, :, off:SQ], ACTF.Exp, scale=0.125
                        )
                        if c >= 4 * j:
                            for t in range(2):
                                nc.gpsimd.tensor_tensor(
                                    att[:, t, off : off + 128],
                                    att[:, t, off : off + 128],
                                    tri_sb[:],
                                    ALU.mult,
                                )
                        if dbg and j == 0 and p == 0 and c == 0:
                            nc.sync.dma_start(out=att_dbg[:], in_=att[:])
                        for t in range(2):
                            hi = 2 * p + t
                            nc.tensor.matmul(
                                avs[t][0:65, off:SQ],
                                v_sb[:, c, 65 * hi : 65 * hi + 65],
                                att[:, t, off:SQ],
                                start=(c == 0),
                                stop=(c == nkb - 1),
                                skip_group_check=True,
                            )
                    if dbg and j == 0 and p == 0:
                        av_cp = otp.tile([65, SQ], f32, tag="ot", name="av_cp")
                        nc.vector.tensor_copy(av_cp[:], avs[0][:])
                        nc.sync.dma_start(out=av_dbg[:], in_=av_cp[:])
                    for t in range(2):
                        hi = 2 * p + t
                        yu = scrp.tile([65, SQ], f32, tag="scr", name="yu")
                        den0 = scrp.tile([1, SQ], f32, tag="scr", name="den0")
                        rec_f = scrp.tile([1, SQ], f32, tag="scr", name="rec_f")
                        rec_r = scrp.tile([1, SQ], f32r, tag="scr", name="rec_r")
                        nc.scalar.copy(yu[:], avs[t][:])
                        nc.vector.tensor_copy(den0[:], avs[t][64:65, :])
                        nc.vector.reciprocal_approx_fast(rec_f[:], den0[:])
                        nc.vector.tensor_copy(rec_r[:], rec_f[:])
                        bc = avp.tile([64, SQ], f32, tag="av", name="bc")
                        nc.tensor.matmul(bc[:], ones1[:], rec_r[:], start=True, stop=True)
                        nc.vector.tensor_mul(
                            yt[64 * (hi % 2) : 64 * (hi % 2) + 64, hi // 2, :],
                            yu[0:64, :],
                            bc[:],
                        )
                        if dbg and j == 0 and p == 0 and t == 0:
                            nc.sync.dma_start(out=rec_dbg[:], in_=rec64[:])

                if dbg and j == 0:
                    nc.sync.dma_start(out=yt_dbg[:], in_=yt[:])
                # ---- phase C: partial projection ------------------------
                for co in range(8):
                    pj = psp.tile([128, SQ], f32, tag="ps", name="pj")
                    for cin in range(4):
                        nc.tensor.matmul(
                            pj[:],
                            wp_sb[:, cin, 128 * co : 128 * co + 128],
                            yt[:, cin, :],
                            start=(cin == 0),
                            stop=(cin == 3),
                        )
                    ot = otp.tile([128, SQ], f32, tag="ot")
                    nc.vector.tensor_copy(ot[:], pj[:])
                    nc.sync.dma_start(
                        out=o_d[128 * co : 128 * co + 128, s0 : s0 + SQ],
                        in_=ot[:],
                    )

        if dbg:
            nc.sync.dma_start(out=kt2_dbg[:], in_=kt2[:])
            nc.sync.dma_start(out=v_dbg[:], in_=v_sb[:])

    nc.compile()
    _BUILD_CACHE["nc"] = nc
    return nc


def _host_prep(x, W_qkv, b_qkv, W_proj):
    """Build the 8 per-core input maps."""
    inv_freq = (1.0 / (ROPE_BASE ** (np.arange(0, D, 2, dtype=np.float32) / D))).astype(
        np.float32
    )
    pos = np.arange(S, dtype=np.float32)
    freqs = np.outer(pos, inv_freq).astype(np.float32)  # [S, 32]
    emb = np.concatenate([freqs, freqs], axis=-1)  # [S, 64]
    cos_t = np.cos(emb).astype(np.float32)
    sin_t = np.sin(emb).astype(np.float32)
    m = np.arange(32)
    ce = np.ascontiguousarray(cos_t[:, 2 * m].T)  # [32, S]
    se = np.ascontiguousarray(sin_t[:, 2 * m].T)
    co = np.ascontiguousarray(cos_t[:, 2 * m + 1].T)
    so = np.ascontiguousarray(sin_t[:, 2 * m + 1].T)
    rope = np.stack([np.tile(t, (4, 1)) for t in (ce, se, co, so)], axis=0)

    qq = np.arange(128)[None, :]
    rr = np.arange(128)[:, None]
    tri = (rr <= qq).astype(ml_dtypes.bfloat16)

    in_maps = []
    for core in range(NC):
        b, g = core // 2, core % 2
        heads = list(range(8 * g, 8 * g + 8))
        cols_e = [64 * heads[4 * c4 + i] + 2 * mm
                  for c4 in range(2) for i in range(4) for mm in range(32)]
        cols_o = [64 * heads[4 * c4 + i] + 2 * mm + 1
                  for c4 in range(2) for i in range(4) for mm in range(32)]
        pi = np.array(cols_e + cols_o)
        vcols = np.array([64 * h + d for h in heads for d in range(64)])

        wq = np.ascontiguousarray(W_qkv[:, pi]).astype(ml_dtypes.bfloat16)
        wk = np.ascontiguousarray(W_qkv[:, C + pi]).astype(ml_dtypes.bfloat16)
        wv = np.ascontiguousarray(W_qkv[:, 2 * C + vcols]).astype(ml_dtypes.bfloat16)
        wp = np.ascontiguousarray(W_proj[vcols, :]).astype(ml_dtypes.bfloat16)
        bq = np.ascontiguousarray(b_qkv[pi].reshape(4, 128).T)
        bk = np.ascontiguousarray(b_qkv[C + pi].reshape(4, 128).T)
        bvr = np.tile(b_qkv[2 * C + vcols][None, :], (128, 1))
        xt = np.ascontiguousarray(x[b].T).astype(ml_dtypes.bfloat16)
        in_maps.append(
            dict(xt=xt, wq=wq, wk=wk, wv=wv, wp=wp, bq=bq, bk=bk,
                 bvr=np.ascontiguousarray(bvr), rope=rope, tri=tri)
        )
    return in_maps


def kernel(x, W_qkv, b_qkv, W_proj, b_proj):
    from concourse.bass_utils import run_bass_kernel_spmd
    import concourse.bass_utils as bass_utils

    x = np.asarray(x, dtype=np.float32)
    W_qkv = np.asarray(W_qkv, dtype=np.float32)
    b_qkv = np.asarray(b_qkv, dtype=np.float32)
    W_proj = np.asarray(W_proj, dtype=np.float32)
    b_proj = np.asarray(b_proj, dtype=np.float32)

    trace = bool(os.environ.get("BASS_KERNEL_TRACE"))
    if trace:
        _install_ntff_shim()
        bass_utils.upload_artifacts = lambda tmpdir: "local://" + tmpdir

    nc = _build()
    in_maps = _host_prep(x, W_qkv, b_qkv, W_proj)
    kw = {}
    if trace:
        tdir = os.environ.get("BASS_KERNEL_TRACE_DIR", "/tmp/ksa_trace")
        os.makedirs(tdir, exist_ok=True)
        kw = dict(trace=True, tmpdir=tdir)
    try:
        res = run_bass_kernel_spmd(nc, in_maps, core_ids=list(range(NC)), **kw)
    except Exception:
        # transient NRT_EXEC_UNIT_UNRECOVERABLE has been observed on the
        # first execution after a fresh compile; one retry clears it
        res = run_bass_kernel_spmd(nc, in_maps, core_ids=list(range(NC)), **kw)
    _BUILD_CACHE["last_result"] = res

    out = np.empty((B, S, C), np.float32)
    for b in range(B):
        oT = res.results[2 * b]["o"] + res.results[2 * b + 1]["o"]
        out[b] = oT.T + b_proj[None, :]
    return out
